# revision 23
# baseline (speedup 1.0000x reference)
"""NonLocalAttention Trainium2 kernel (row-tiled QK, exp-saturated softmax).

Reference computation (N=2, C=64, CR=32, H=W=96, HW=9216):
    e1  = PReLU(w1 @ inputa + b1)   # [N,32,HW]   (queries)
    e2  = PReLU(w2 @ inputb + b2)   # [N,32,HW]   (keys)
    asm = PReLU(wa @ inputa + ba)   # [N,64,HW]   (values)
    out = softmax(e1^T e2, axis=keys) @ asm^T + inputa
Sharding: 8 cores = 2 batches x 4 query-chunks of 2304 rows. Softmax is
key-order invariant, so the host ROTATES the key/value columns per core so
that each core's query chunk is always columns 0:2304 — one SPMD program,
no per-core offsets. No collectives.

The kernel is EXP-BOUND: 21.2M score elements per core pass through the
scalar (ACT) engine at 1 elem/cycle/lane @1.2GHz with ~190cy fixed cost per
ACTIVATE — a ~155us floor at [128,1536] activations. Everything else hides
under that chain:
  - QK is ROW-TILED (3 concurrent K=32 matmuls in PE row groups, one PSUM
    bank each); e1/e2/v layouts and the ones-row bias/denominator tricks as
    before (conv contract K=65; PV emits the softmax denominator as row 64).
  - PSUM banking: score claims 2x[128,1536] (6 banks) + po 1 bank + a
    dedicated 1-bank FILLER slot. Deferred preamble work (e2 blocks 3-23,
    v tiles, e1 blocks 1-4) runs in the filler slot so it never perturbs
    the QK/exp claim parity (a filler claim in the score pool would force
    the next QK onto the buffer exp is still reading - ~0.5us stall each).
  - fillers are sliced to < 1 exp of PE work and scheduled against their
    deadlines: v in 6-tile units on even block-0 iterations (one PSUM
    group per unit, like the old emit_v), e2 in 2-colblock units on odd
    iterations (row-tiled trios, bank-sharing across partitions is safe),
    e1 blocks in free late iterations.
  - PReLU is two DVE ops (mul + max; walrus rejects the fused
    scalar_tensor_tensor form and the Pool engine cannot run max or
    touch PSUM, so this stays on the DVE).
  - the 256-query tail block packs SIX key tiles per claim (two bank-safe
    rounds: bank b holds the two tiles whose row group is b, so same-bank
    tiles serialize on their row group) -> 12 full [128,1536] exps instead
    of 24 half-empty ones.
  - preamble: all three weight matrices ride ONE DMA (w1r|w2a|waa
    concatenated as [65,224]); PE warms up on a memset dummy tile with no
    DMA dependency; first exp needs only e1 block 0 + e2 colblocks 0-2;
    input DMAs are split across the sync + gpsimd queues in consumer order
    (each trigger costs ~0.7us of queue time, so triggers are few and
    parallel).
  - block epilogues FIRST copy po to SBUF in one DVE op (po is single
    buffered; the next block's PV gates on that read), then defer the
    reciprocal / broadcast / mul / add into scheduled slots of the next
    block's iteration stream so the DMA-gated ops never jam the in-order
    DVE queue. The final block broadcasts the reciprocal row through an
    idle-PE K=1 matmul in halves. (reciprocal_approx_fast faults this
    NRT - custom-DVE uop tables don't load on the axon path - and Pool
    compute triggers a ~17% core downclock; both are avoided.)
  - scores are bounded well below exp overflow, so no max-subtraction;
    everything on the PE is bf16, PSUM stays fp32.
"""

import os

import numpy as np

RECIP_FAST = os.environ.get("K_RECIP_FAST", "0") == "1"
TAIL_PACK = os.environ.get("K_TAIL_PACK", "1") == "1"

C = 64
CR = 32
HW = 9216
QCH = 2304  # query rows per core
NKT = HW // 128  # 72 key tiles
R = 3  # row-tiled QK tiles per group
NG = NKT // R  # 24 key groups
NCORES = 8
NMAIN = 4  # 4 main q-blocks of 512 + tail of 256
TAIL_OFF = 2048
TAIL_NQ = 256


def _ensure_ntff_hook():
    """Best-effort registration of the axon NTFF profile hook; the agent
    image's antenv package lacks axon_hooks, which would make any traced
    run crash on import instead of degrading."""
    import sys
    import types

    try:
        import antenv.axon_hooks  # noqa: F401

        return
    except ImportError:
        pass
    try:
        import antenv
        from trn_agent_boot.trn_boot import _ntff_profile_via_ctypes

        hook = _ntff_profile_via_ctypes("/opt/axon/libaxon_pjrt.so")
        mod = types.ModuleType("antenv.axon_hooks")
        _h = [hook]
        mod.get_axon_ntff_profile_hook = lambda: _h[0]
        mod.set_axon_ntff_profile_hook = lambda h: _h.__setitem__(0, h)
        sys.modules["antenv.axon_hooks"] = mod
        antenv.axon_hooks = mod
    except Exception:
        pass


def build_program(a1: float, a2: float, aa: float):
    import concourse.bacc as bacc
    import concourse.tile as tile
    from concourse import mybir

    f32 = mybir.dt.float32
    bf16 = mybir.dt.bfloat16
    AF = mybir.ActivationFunctionType
    OP = mybir.AluOpType

    nc = bacc.Bacc()
    xa = nc.dram_tensor("xa", [C + 1, HW], bf16, kind="ExternalInput")
    xb = nc.dram_tensor("xb", [C + 1, HW], bf16, kind="ExternalInput")
    # wts = w1r (4-replicated w1^T+b1, 128) | w2a (32) | waa (64)
    wts = nc.dram_tensor("wts", [C + 1, 224], bf16, kind="ExternalInput")
    out = nc.dram_tensor("out", [C, QCH], f32, kind="ExternalOutput")

    prelu_pool = [None]  # set to the work pool inside the context

    def recip(outp, inp):
        if RECIP_FAST:
            nc.vector.reciprocal_approx_fast(outp, inp)
        else:
            nc.vector.reciprocal(outp, inp)

    def prelu(outp, in0, a, pool_eng=False):
        # PReLU: out = max(a*x, x) (walrus rejects the fused
        # scalar_tensor_tensor form). Default: two DVE ops straight from
        # PSUM. pool_eng=True: one DVE copy PSUM->SBUF (frees the PSUM
        # claim fast), then mul+max on the idle Pool engine (SBUF-only -
        # GPSIMD cannot touch PSUM). Keeps filler PReLUs off the DVE
        # queue where epilogue ops would jam them.
        # (Pool cannot run max or touch PSUM, so PReLU is DVE-only.)
        p, n = in0.shape
        scr = prelu_pool[0].tile([128, 512], f32, tag="scr")
        nc.vector.tensor_scalar_mul(scr[0:p, 0:n], in0, float(a))
        nc.vector.tensor_max(outp, scr[0:p, 0:n], in0)

    with tile.TileContext(nc) as tc:
        with (
            tc.tile_pool(name="consts", bufs=1) as consts,
            tc.tile_pool(name="big", bufs=1) as big,
            tc.tile_pool(name="ps", bufs=2, space="PSUM") as ps,
            tc.tile_pool(name="po", bufs=1, space="PSUM") as ps_o,
            tc.tile_pool(name="fill", bufs=1, space="PSUM") as ps_f,
            tc.tile_pool(name="pt", bufs=4) as ptile,
            tc.tile_pool(name="work", bufs=2) as work,
        ):
            prelu_pool[0] = work
            # --- constants / dummies --------------------------------------
            dummy = consts.tile([128, 128], bf16, tag="dummy")
            nc.vector.memset(dummy[:], 1.0)
            ones_sb = consts.tile([1, C], f32, tag="ones")
            nc.vector.memset(ones_sb[:], 1.0)

            wts_sb = consts.tile([C + 1, 224], bf16, tag="wts")
            w1r_sb = wts_sb[:, 0:128]
            w2a_sb = wts_sb[:, 128:160]
            waa_sb = wts_sb[:, 160:224]

            xa_sb = big.tile([C + 1, HW], bf16, tag="xa")
            xb_sb = big.tile([C + 1, HW], bf16, tag="xb")

            # ALL input DMAs on one queue, in consumer-priority order: the
            # DMA engines drain the queue FIFO, so the critical preamble
            # slices land first (a second trigger queue would race the bulk
            # transfers ahead of them on the shared engines).
            # ALL input DMAs on one queue, in consumer-priority order: the
            # DMA engines drain the queue FIFO, so the critical preamble
            # slices land first (a second trigger queue would race the bulk
            # transfers ahead of them on the shared engines).
            nc.sync.dma_start(wts_sb[:], wts[:])
            nc.sync.dma_start(xa_sb[:, 0:512], xa[:, 0:512])  # e1 block 0
            nc.sync.dma_start(xb_sb[:, 0:1152], xb[:, 0:1152])  # e2 blocks 0-2
            nc.sync.dma_start(xa_sb[:, 512:1280], xa[:, 512:1280])  # v unit 0-1
            nc.sync.dma_start(xb_sb[:, 1152:1920], xb[:, 1152:1920])  # e2 u0
            nc.sync.dma_start(xa_sb[:, 1280:2560], xa[:, 1280:2560])
            nc.sync.dma_start(xb_sb[:, 1920:3072], xb[:, 1920:3072])
            nc.sync.dma_start(xa_sb[:, 2560:4608], xa[:, 2560:4608])
            nc.sync.dma_start(xb_sb[:, 3072:5376], xb[:, 3072:5376])
            nc.sync.dma_start(xa_sb[:, 4608:HW], xa[:, 4608:HW])
            nc.sync.dma_start(xb_sb[:, 5376:HW], xb[:, 5376:HW])

            e1_sb = big.tile([128, QCH], bf16, tag="e1")
            e2_sb = big.tile([96, NG * 128], bf16, tag="e2")
            v_all = big.tile([128, NKT * 65], bf16, tag="vall")
            v3 = v_all[:].rearrange("p (t c) -> p t c", c=65)
            nc.vector.memset(v3[:, :, 64:65], 1.0)

            # --- PE warmup on the dummy tile: no DMA dependency, so the
            # HAM clock gate ramps to 2.4GHz while the inputs stream in.
            warm = ps_f.tile([128, 512], f32, tag="fill")
            for wq in range(14):
                nc.tensor.matmul(
                    warm[:, 0:128], dummy[:], dummy[:], start=True, stop=True
                )

            # --- preamble compute: exactly what exp(0)..exp(2) need ------
            # e1 block 0 (queries 0:512), replicated in 4 row groups
            pse1 = ps.tile([128, 1536], f32, tag="ps")
            nc.tensor.matmul(
                pse1[:, 0:512], w1r_sb, xa_sb[:, 0:512], start=True, stop=True
            )
            # native Prelu on the idle ACT engine: removes the serial DVE
            # mul+max pair from the first-exp critical path (Prelu is in
            # the exp table sets, so no extra table load)
            nc.scalar.activation(
                e1_sb[:, 0:512], pse1[:, 0:512], AF.Prelu, alpha=float(a1)
            )

            # e2 col blocks 0-2 (key tiles 0-8): tile kt at partitions
            # 32*(kt%3), col block kt//3
            pse2 = ps.tile([128, 1536], f32, tag="ps")
            for kt in range(9):
                nc.tensor.matmul(
                    pse2[32 * (kt % 3) : 32 * (kt % 3 + 1),
                         (kt // 3) * 128 : (kt // 3 + 1) * 128],
                    w2a_sb,
                    xb_sb[:, kt * 128 : (kt + 1) * 128],
                    start=True, stop=True,
                )
            nc.scalar.activation(
                e2_sb[:, 0:128], pse2[0:96, 0:128], AF.Prelu, alpha=float(a2)
            )  # QK(0) gate
            nc.scalar.activation(
                e2_sb[:, 128:384], pse2[0:96, 128:384], AF.Prelu, alpha=float(a2)
            )

            # --- filler units (run in the dedicated 1-bank PSUM slot) ----
            def fill_v(w):
                # v tiles 6w..6w+5: one accumulation group per unit so the
                # bank never holds two open matmul groups
                psv = ps_f.tile([128, 512], f32, tag="fill")
                for i in range(6):
                    kt = 6 * w + i
                    nc.tensor.matmul(
                        psv[:, i * 64 : (i + 1) * 64],
                        xa_sb[:, kt * 128 : (kt + 1) * 128],
                        waa_sb,
                        start=(i == 0), stop=(i == 5),
                    )
                psv3 = psv[:, 0:384].rearrange("p (t c) -> p t c", c=64)
                scr = work.tile([128, 512], f32, tag="scr")
                nc.vector.tensor_scalar_mul(scr[:, 0:384], psv[:, 0:384], float(aa))
                scr3 = scr[:, 0:384].rearrange("p (t c) -> p t c", c=64)
                nc.vector.tensor_max(v3[:, 6 * w : 6 * w + 6, 0:64], scr3, psv3)

            def fill_e2(b0, nb):
                # e2 col blocks b0..b0+nb-1 (row-tiled trios; different
                # partitions may share the bank)
                pse = ps_f.tile([128, 512], f32, tag="fill")
                for m in range(nb):
                    for j in range(R):
                        kt = R * (b0 + m) + j
                        nc.tensor.matmul(
                            pse[32 * j : 32 * (j + 1), m * 128 : (m + 1) * 128],
                            w2a_sb,
                            xb_sb[:, kt * 128 : (kt + 1) * 128],
                            start=True, stop=True,
                        )
                prelu(
                    e2_sb[:, b0 * 128 : (b0 + nb) * 128],
                    pse[0:96, 0 : nb * 128],
                    a2,
                    pool_eng=True,
                )

            def fill_e1(off, n):
                pse = ps_f.tile([128, 512], f32, tag="fill")
                nc.tensor.matmul(
                    pse[:, 0:n], w1r_sb, xa_sb[:, off : off + n],
                    start=True, stop=True,
                )
                prelu(e1_sb[:, off : off + n], pse[:, 0:n], a1, pool_eng=True)

            # schedule: global iteration t (24 per main block).
            # v units (needed by PV in block 0) on evens, e2 units (needed
            # by QK in block 0) on odds, e1 blocks in the free late slots.
            # v unit 0 runs in the preamble shadow so the in-loop filler
            # chain (claim -> prelu -> next claim) starts pre-warmed
            fill_v(0)

            fillers = {}
            for w in range(1, 12):
                fillers.setdefault(2 * w - 2, []).append(lambda w=w: fill_v(w))
            for u in range(11):
                nb = 2 if u < 10 else 1
                fillers.setdefault(2 * u + 1, []).append(
                    lambda u=u, nb=nb: fill_e2(3 + 2 * u, nb)
                )
            # e1 blocks spread clear of the block-boundary DVE crunch
            fillers.setdefault(23, []).append(lambda: fill_e1(512, 512))
            fillers.setdefault(27, []).append(lambda: fill_e1(1024, 512))
            fillers.setdefault(31, []).append(lambda: fill_e1(1536, 512))
            fillers.setdefault(35, []).append(lambda: fill_e1(2048, 256))

            # --- main attention blocks (nq=512), software-pipelined ------
            for bi in range(NMAIN):
                off = 512 * bi
                po = ps_o.tile([C + 1, 512], f32, tag="po")
                pt_prev = None
                g_prev = -1

                def emit_pv(ptx, gx, po=po):
                    for j in range(R):
                        kt = R * gx + j
                        nc.tensor.matmul(
                            po[:, 0:512],
                            v_all[:, kt * 65 : (kt + 1) * 65],
                            ptx[:, j * 512 : (j + 1) * 512],
                            start=(kt == 0), stop=(kt == NKT - 1),
                        )

                for g in range(NG):
                    t = bi * NG + g
                    pss = ps.tile([128, 1536], f32, tag="ps")
                    for j in range(R):
                        kt = R * g + j
                        nc.tensor.matmul(
                            pss[:, j * 512 : (j + 1) * 512],
                            e2_sb[
                                32 * (kt % 3) : 32 * (kt % 3 + 1),
                                (kt // 3) * 128 : (kt // 3 + 1) * 128,
                            ],
                            e1_sb[32 * (kt % 3) : 32 * (kt % 3 + 1), off : off + 512],
                            start=True, stop=True,
                        )
                    pt = ptile.tile([128, 1536], bf16, tag="pt")
                    nc.scalar.activation(pt[:], pss[:], AF.Exp)
                    for fn in fillers.get(t, ()):
                        fn()
                    if pt_prev is not None:
                        emit_pv(pt_prev, g_prev)
                    pt_prev, g_prev = pt, g
                emit_pv(pt_prev, g_prev)

                # epilogue: out = po[0:64]/po[64] + xq. FIRST copy all of
                # po to SBUF in one DVE op - po is single-buffered, so the
                # next block's PV accumulation is gated on this read. The
                # recip + 0-stride broadcast DMA issue now, but the
                # DMA-gated mul and the add/store are DEFERRED into the
                # next block's iteration stream (the DVE queue is in-order,
                # so a waiting mul emitted here would jam the next block's
                # filler PReLUs behind it).
                poc = work.tile([C + 1, 512], f32, tag="poc")
                nc.vector.tensor_copy(poc[:], po[:])
                rec = work.tile([1, 512], f32, tag="rec")
                rb = work.tile([C, 512], f32, tag="rb")
                osb = work.tile([C, 512], f32, tag="osb")

                def epi_rec(rec=rec, poc=poc, rb=rb):
                    recip(rec[0:1, 0:512], poc[C : C + 1, 0:512])
                    rec_rep = rec[0:1, 0:512].rearrange("a (b c) -> a b c", b=1)
                    nc.gpsimd.dma_start(rb[:], rec_rep.to_broadcast((1, C, 512)))

                def epi_mul(osb=osb, rb=rb, poc=poc):
                    nc.vector.tensor_mul(osb[:], rb[:], poc[0:C, 0:512])

                def epi_add(osb=osb, off=off):
                    nc.vector.tensor_add(osb[:], osb[:], xa_sb[0:C, off : off + 512])
                    nc.sync.dma_start(out[:, off : off + 512], osb[:])

                t_next = (bi + 1) * NG
                fillers.setdefault(t_next + 2, []).append(epi_rec)
                fillers.setdefault(t_next + 6, []).append(epi_mul)
                fillers.setdefault(t_next + 8, []).append(epi_add)

            # --- tail block (queries 2048:2304, nq=256): SIX packed key
            # tiles per claim. Bank b holds the two tiles whose row group
            # is b (they serialize on the row group, so the bank never has
            # two concurrent groups); rounds r=0,1 issue as row-tiled trios.
            po = ps_o.tile([C + 1, 512], f32, tag="po")
            pt_prev = None
            sg_prev = -1

            def tail_col(kt):
                if TAIL_PACK:
                    return 512 * (kt % 3) + 256 * ((kt % 6) // 3)
                return 512 * (kt % 3)

            def emit_pv_tail(ptx, sgx):
                tpg = 6 if TAIL_PACK else 3
                for i in range(tpg):
                    kt = tpg * sgx + i
                    c0 = tail_col(kt)
                    nc.tensor.matmul(
                        po[:, 0:256],
                        v_all[:, kt * 65 : (kt + 1) * 65],
                        ptx[:, c0 : c0 + 256],
                        start=(kt == 0), stop=(kt == NKT - 1),
                    )

            NSG = 12 if TAIL_PACK else 24
            TPG = 6 if TAIL_PACK else 3
            for sg in range(NSG):
                pss = ps.tile([128, 1536], f32, tag="ps")
                for i in range(TPG):
                    kt = TPG * sg + i
                    c0 = tail_col(kt)
                    nc.tensor.matmul(
                        pss[:, c0 : c0 + 256],
                        e2_sb[
                            32 * (kt % 3) : 32 * (kt % 3 + 1),
                            (kt // 3) * 128 : (kt // 3 + 1) * 128,
                        ],
                        e1_sb[
                            32 * (kt % 3) : 32 * (kt % 3 + 1),
                            TAIL_OFF : TAIL_OFF + 256,
                        ],
                        start=True, stop=True,
                    )
                pt = ptile.tile([128, 1536], bf16, tag="pt")
                if TAIL_PACK:
                    nc.scalar.activation(pt[:], pss[:], AF.Exp)
                else:
                    pss3 = pss[:].rearrange("p (t c) -> p t c", c=512)
                    pt3 = pt[:].rearrange("p (t c) -> p t c", c=512)
                    nc.scalar.activation(pt3[:, :, 0:256], pss3[:, :, 0:256], AF.Exp)
                for fn in fillers.get(NMAIN * NG + sg, ()):
                    fn()
                if pt_prev is not None:
                    emit_pv_tail(pt_prev, sg_prev)
                pt_prev, sg_prev = pt, sg
            emit_pv_tail(pt_prev, sg_prev)

            # tail epilogue, split in halves so the DVE/PE/DMA chains
            # pipeline; reciprocal broadcast via an idle-PE K=1 matmul
            # (a broadcast DMA would add ~4us of un-hidden queue latency).
            # tail epilogue in halves (one engine does all the work, so
            # finer slicing only adds per-slice fixed costs)
            hn = 128
            rbps = []
            recs = []
            for h in range(2):
                rec = work.tile([1, 512], f32, tag="rec")
                recs.append(rec)
            # h0 on DVE, h1 on the now-idle ACT engine (exp(-ln d); Ln and
            # Exp share the natural_log_exp set, so still one table load) -
            # the two reciprocals run in PARALLEL instead of serializing
            nc.vector.reciprocal(recs[0][0:1, 0:hn], po[C : C + 1, 0:hn])
            lnt = work.tile([1, 512], f32, tag="lnt")
            nc.scalar.activation(lnt[0:1, 0:hn], po[C : C + 1, hn : 2 * hn], AF.Ln)
            nc.scalar.activation(recs[1][0:1, 0:hn], lnt[0:1, 0:hn], AF.Exp, scale=-1.0)
            for h in range(2):
                rec = recs[h]
                rbp = ps.tile([128, 1536], f32, tag="ps")
                nc.tensor.matmul(
                    rbp[0:C, 0:hn], ones_sb[:], rec[0:1, 0:hn],
                    start=True, stop=True,
                )
                rbps.append(rbp)
            for h in range(2):
                ho = h * hn
                rb = work.tile([C, 512], f32, tag="rb")
                nc.vector.tensor_copy(rb[:, 0:hn], rbps[h][0:C, 0:hn])
                osb = work.tile([C, 512], f32, tag="osb")
                nc.vector.tensor_mul(osb[:, 0:hn], rb[:, 0:hn], po[0:C, ho : ho + hn])
                nc.vector.tensor_add(
                    osb[:, 0:hn], osb[:, 0:hn],
                    xa_sb[0:C, TAIL_OFF + ho : TAIL_OFF + ho + hn],
                )
                nc.sync.dma_start(
                    out[:, TAIL_OFF + ho : TAIL_OFF + ho + hn], osb[:, 0:hn]
                )
    nc.finalize()
    return nc


def run(inputs: dict, trace: bool = False, tmpdir: str | None = None):
    """Build, compile and run on 8 cores; returns (output, BassKernelResults)."""
    _ensure_ntff_hook()
    from concourse.bass_utils import run_bass_kernel_spmd

    inputa = np.asarray(inputs["inputa"], dtype=np.float32)
    inputb = np.asarray(inputs["inputb"], dtype=np.float32)
    w1 = np.asarray(inputs["w1"], dtype=np.float32)
    b1 = np.asarray(inputs["b1"], dtype=np.float32)
    w2 = np.asarray(inputs["w2"], dtype=np.float32)
    b2 = np.asarray(inputs["b2"], dtype=np.float32)
    wa = np.asarray(inputs["wa"], dtype=np.float32)
    ba = np.asarray(inputs["ba"], dtype=np.float32)
    a1 = float(np.asarray(inputs["a1"]).reshape(-1)[0])
    a2 = float(np.asarray(inputs["a2"]).reshape(-1)[0])
    aa = float(np.asarray(inputs["aa"]).reshape(-1)[0])

    N, Cc, H, W = inputa.shape
    assert (N, Cc, H * W) == (2, C, HW), inputa.shape
    chunks_per_batch = NCORES // N  # 4

    import ml_dtypes

    bf = ml_dtypes.bfloat16

    xa_n = inputa.reshape(N, C, HW)
    xb_n = inputb.reshape(N, C, HW)

    def aug65(x):
        """[64, HW] -> [65, HW] bf16 with a ones row at 64."""
        p = np.empty((C + 1, x.shape[1]), np.float32)
        p[:C] = x
        p[C] = 1.0
        return p.astype(bf)

    def wpad(wt, b, rep=1):
        """[64, M] weights^T + bias row at 64; optional column replication
        for the row-tiled QK stationary layout."""
        m = wt.shape[1]
        p = np.empty((C + 1, m * rep), np.float32)
        for r in range(rep):
            p[:C, r * m : (r + 1) * m] = wt
            p[C, r * m : (r + 1) * m] = b
        return p.astype(bf)

    wts_aug = np.concatenate(
        [wpad(w1.T, b1, rep=4), wpad(w2.T, b2), wpad(wa.T, ba)], axis=1
    )  # [65, 224]

    in_maps = []
    for core in range(NCORES):
        b, chunk = divmod(core, chunks_per_batch)
        qoff = chunk * QCH
        # rotate keys/values so this core's queries are columns 0:QCH
        # (softmax over keys is invariant to the key order)
        rot = np.concatenate([xa_n[b][:, qoff:], xa_n[b][:, :qoff]], axis=1)
        rot_b = np.concatenate([xb_n[b][:, qoff:], xb_n[b][:, :qoff]], axis=1)
        in_maps.append({"xa": aug65(rot), "xb": aug65(rot_b), "wts": wts_aug})

    nc = build_program(a1, a2, aa)
    res = run_bass_kernel_spmd(
        nc, in_maps, list(range(NCORES)), trace=trace, tmpdir=tmpdir
    )

    out = np.empty((N, C, HW), np.float32)
    for core in range(NCORES):
        b, chunk = divmod(core, chunks_per_batch)
        out[b, :, chunk * QCH : (chunk + 1) * QCH] = res.results[core]["out"]
    return out.reshape(N, C, H, W), res


def kernel(**inputs) -> np.ndarray:
    out, _ = run(inputs, trace=False)
    return out


# revision 24
# speedup vs baseline: 1.0110x; 1.0110x over previous
"""NonLocalAttention Trainium2 kernel (row-tiled QK, exp-saturated softmax).

Reference computation (N=2, C=64, CR=32, H=W=96, HW=9216):
    e1  = PReLU(w1 @ inputa + b1)   # [N,32,HW]   (queries)
    e2  = PReLU(w2 @ inputb + b2)   # [N,32,HW]   (keys)
    asm = PReLU(wa @ inputa + ba)   # [N,64,HW]   (values)
    out = softmax(e1^T e2, axis=keys) @ asm^T + inputa
Sharding: 8 cores = 2 batches x 4 query-chunks of 2304 rows. Softmax is
key-order invariant, so the host ROTATES the key/value columns per core so
that each core's query chunk is always columns 0:2304 — one SPMD program,
no per-core offsets. No collectives.

The kernel is EXP-BOUND: 21.2M score elements per core pass through the
scalar (ACT) engine at 1 elem/cycle/lane @1.2GHz with ~190cy fixed cost per
ACTIVATE — a ~155us floor at [128,1536] activations. Everything else hides
under that chain:
  - QK is ROW-TILED (3 concurrent K=32 matmuls in PE row groups, one PSUM
    bank each); e1/e2/v layouts and the ones-row bias/denominator tricks as
    before (conv contract K=65; PV emits the softmax denominator as row 64).
  - PSUM banking: score claims 2x[128,1536] (6 banks) + po 1 bank + a
    dedicated 1-bank FILLER slot. Deferred preamble work (e2 blocks 3-23,
    v tiles, e1 blocks 1-4) runs in the filler slot so it never perturbs
    the QK/exp claim parity (a filler claim in the score pool would force
    the next QK onto the buffer exp is still reading - ~0.5us stall each).
  - fillers are sliced to < 1 exp of PE work and scheduled against their
    deadlines: v in 6-tile units on even block-0 iterations (one PSUM
    group per unit, like the old emit_v), e2 in 2-colblock units on odd
    iterations (row-tiled trios, bank-sharing across partitions is safe),
    e1 blocks in free late iterations.
  - PReLU is two DVE ops (mul + max; walrus rejects the fused
    scalar_tensor_tensor form and the Pool engine cannot run max or
    touch PSUM, so this stays on the DVE).
  - the 256-query tail block packs SIX key tiles per claim (two bank-safe
    rounds: bank b holds the two tiles whose row group is b, so same-bank
    tiles serialize on their row group) -> 12 full [128,1536] exps instead
    of 24 half-empty ones.
  - preamble: all three weight matrices ride ONE DMA (w1r|w2a|waa
    concatenated as [65,224]); PE warms up on a memset dummy tile with no
    DMA dependency; first exp needs only e1 block 0 + e2 colblocks 0-2;
    input DMAs are split across the sync + gpsimd queues in consumer order
    (each trigger costs ~0.7us of queue time, so triggers are few and
    parallel).
  - block epilogues FIRST copy po to SBUF in one DVE op (po is single
    buffered; the next block's PV gates on that read), then defer the
    reciprocal / broadcast / mul / add into scheduled slots of the next
    block's iteration stream so the DMA-gated ops never jam the in-order
    DVE queue. The final block broadcasts the reciprocal row through an
    idle-PE K=1 matmul in halves. (reciprocal_approx_fast faults this
    NRT - custom-DVE uop tables don't load on the axon path - and Pool
    compute triggers a ~17% core downclock; both are avoided.)
  - scores are bounded well below exp overflow, so no max-subtraction;
    everything on the PE is bf16, PSUM stays fp32.
"""

import os

import numpy as np

RECIP_FAST = os.environ.get("K_RECIP_FAST", "0") == "1"
TAIL_PACK = os.environ.get("K_TAIL_PACK", "1") == "1"

C = 64
CR = 32
HW = 9216
QCH = 2304  # query rows per core
NKT = HW // 128  # 72 key tiles
R = 3  # row-tiled QK tiles per group
NG = NKT // R  # 24 key groups
NCORES = 8
NMAIN = 4  # 4 main q-blocks of 512 + tail of 256
TAIL_OFF = 2048
TAIL_NQ = 256


def _ensure_ntff_hook():
    """Best-effort registration of the axon NTFF profile hook; the agent
    image's antenv package lacks axon_hooks, which would make any traced
    run crash on import instead of degrading."""
    import sys
    import types

    try:
        import antenv.axon_hooks  # noqa: F401

        return
    except ImportError:
        pass
    try:
        import antenv
        from trn_agent_boot.trn_boot import _ntff_profile_via_ctypes

        hook = _ntff_profile_via_ctypes("/opt/axon/libaxon_pjrt.so")
        mod = types.ModuleType("antenv.axon_hooks")
        _h = [hook]
        mod.get_axon_ntff_profile_hook = lambda: _h[0]
        mod.set_axon_ntff_profile_hook = lambda h: _h.__setitem__(0, h)
        sys.modules["antenv.axon_hooks"] = mod
        antenv.axon_hooks = mod
    except Exception:
        pass


def build_program(a1: float, a2: float, aa: float):
    import concourse.bacc as bacc
    import concourse.tile as tile
    from concourse import mybir

    f32 = mybir.dt.float32
    bf16 = mybir.dt.bfloat16
    AF = mybir.ActivationFunctionType
    OP = mybir.AluOpType

    nc = bacc.Bacc()
    xa = nc.dram_tensor("xa", [C + 1, HW], bf16, kind="ExternalInput")
    xb = nc.dram_tensor("xb", [C + 1, HW], bf16, kind="ExternalInput")
    # wts = w1r (4-replicated w1^T+b1, 128) | w2a (32) | waa (64)
    wts = nc.dram_tensor("wts", [C + 1, 224], bf16, kind="ExternalInput")
    out = nc.dram_tensor("out", [C, QCH], f32, kind="ExternalOutput")

    prelu_pool = [None]  # set to the work pool inside the context

    def recip(outp, inp):
        if RECIP_FAST:
            nc.vector.reciprocal_approx_fast(outp, inp)
        else:
            nc.vector.reciprocal(outp, inp)

    def prelu(outp, in0, a, pool_eng=False):
        # PReLU: out = max(a*x, x) (walrus rejects the fused
        # scalar_tensor_tensor form). Default: two DVE ops straight from
        # PSUM. pool_eng=True: one DVE copy PSUM->SBUF (frees the PSUM
        # claim fast), then mul+max on the idle Pool engine (SBUF-only -
        # GPSIMD cannot touch PSUM). Keeps filler PReLUs off the DVE
        # queue where epilogue ops would jam them.
        # (Pool cannot run max or touch PSUM, so PReLU is DVE-only.)
        p, n = in0.shape
        scr = prelu_pool[0].tile([128, 512], f32, tag="scr")
        nc.vector.tensor_scalar_mul(scr[0:p, 0:n], in0, float(a))
        nc.vector.tensor_max(outp, scr[0:p, 0:n], in0)

    with tile.TileContext(nc) as tc:
        with (
            tc.tile_pool(name="consts", bufs=1) as consts,
            tc.tile_pool(name="big", bufs=1) as big,
            tc.tile_pool(name="ps", bufs=2, space="PSUM") as ps,
            tc.tile_pool(name="po", bufs=1, space="PSUM") as ps_o,
            tc.tile_pool(name="fill", bufs=1, space="PSUM") as ps_f,
            tc.tile_pool(name="pt", bufs=4) as ptile,
            tc.tile_pool(name="work", bufs=2) as work,
        ):
            prelu_pool[0] = work
            # --- constants / dummies --------------------------------------
            dummy = consts.tile([128, 128], bf16, tag="dummy")
            nc.vector.memset(dummy[:], 1.0)
            ones_sb = consts.tile([1, C], f32, tag="ones")
            nc.vector.memset(ones_sb[:], 1.0)

            wts_sb = consts.tile([C + 1, 224], bf16, tag="wts")
            w1r_sb = wts_sb[:, 0:128]
            w2a_sb = wts_sb[:, 128:160]
            waa_sb = wts_sb[:, 160:224]

            xa_sb = big.tile([C + 1, HW], bf16, tag="xa")
            xb_sb = big.tile([C + 1, HW], bf16, tag="xb")

            # ALL input DMAs on one queue, in consumer-priority order: the
            # DMA engines drain the queue FIFO, so the critical preamble
            # slices land first (a second trigger queue would race the bulk
            # transfers ahead of them on the shared engines).
            # ALL input DMAs on one queue, in consumer-priority order: the
            # DMA engines drain the queue FIFO, so the critical preamble
            # slices land first (a second trigger queue would race the bulk
            # transfers ahead of them on the shared engines).
            nc.sync.dma_start(wts_sb[:], wts[:])
            nc.sync.dma_start(xa_sb[:, 0:512], xa[:, 0:512])  # e1 block 0
            nc.sync.dma_start(xb_sb[:, 0:1152], xb[:, 0:1152])  # e2 blocks 0-2
            nc.sync.dma_start(xa_sb[:, 512:1280], xa[:, 512:1280])  # v unit 0-1
            nc.sync.dma_start(xb_sb[:, 1152:1920], xb[:, 1152:1920])  # e2 u0
            nc.sync.dma_start(xa_sb[:, 1280:2560], xa[:, 1280:2560])
            nc.sync.dma_start(xb_sb[:, 1920:3072], xb[:, 1920:3072])
            nc.sync.dma_start(xa_sb[:, 2560:4608], xa[:, 2560:4608])
            nc.sync.dma_start(xb_sb[:, 3072:5376], xb[:, 3072:5376])
            nc.sync.dma_start(xa_sb[:, 4608:HW], xa[:, 4608:HW])
            nc.sync.dma_start(xb_sb[:, 5376:HW], xb[:, 5376:HW])

            e1_sb = big.tile([128, QCH], bf16, tag="e1")
            e2_sb = big.tile([96, NG * 128], bf16, tag="e2")
            v_all = big.tile([128, NKT * 65], bf16, tag="vall")
            v3 = v_all[:].rearrange("p (t c) -> p t c", c=65)
            nc.vector.memset(v3[:, :, 64:65], 1.0)

            # --- PE warmup on the dummy tile: no DMA dependency, so the
            # HAM clock gate ramps to 2.4GHz while the inputs stream in.
            warm = ps_f.tile([128, 512], f32, tag="fill")
            for wq in range(14):
                nc.tensor.matmul(
                    warm[:, 0:128], dummy[:], dummy[:], start=True, stop=True
                )

            # --- preamble compute: exactly what exp(0)..exp(2) need ------
            # e1 block 0 (queries 0:512), replicated in 4 row groups
            pse1 = ps.tile([128, 1536], f32, tag="ps")
            nc.tensor.matmul(
                pse1[:, 0:512], w1r_sb, xa_sb[:, 0:512], start=True, stop=True
            )
            # native Prelu on the idle ACT engine: removes the serial DVE
            # mul+max pair from the first-exp critical path (Prelu is in
            # the exp table sets, so no extra table load)
            nc.scalar.activation(
                e1_sb[:, 0:512], pse1[:, 0:512], AF.Prelu, alpha=float(a1)
            )

            # e2 col blocks 0-2 (key tiles 0-8): tile kt at partitions
            # 32*(kt%3), col block kt//3
            pse2 = ps.tile([128, 1536], f32, tag="ps")
            for kt in range(9):
                nc.tensor.matmul(
                    pse2[32 * (kt % 3) : 32 * (kt % 3 + 1),
                         (kt // 3) * 128 : (kt // 3 + 1) * 128],
                    w2a_sb,
                    xb_sb[:, kt * 128 : (kt + 1) * 128],
                    start=True, stop=True,
                )
            nc.scalar.activation(
                e2_sb[:, 0:128], pse2[0:96, 0:128], AF.Prelu, alpha=float(a2)
            )  # QK(0) gate
            nc.scalar.activation(
                e2_sb[:, 128:384], pse2[0:96, 128:384], AF.Prelu, alpha=float(a2)
            )

            # --- filler units (run in the dedicated 1-bank PSUM slot) ----
            def fill_v(w):
                # v tiles 6w..6w+5: one accumulation group per unit so the
                # bank never holds two open matmul groups
                psv = ps_f.tile([128, 512], f32, tag="fill")
                for i in range(6):
                    kt = 6 * w + i
                    nc.tensor.matmul(
                        psv[:, i * 64 : (i + 1) * 64],
                        xa_sb[:, kt * 128 : (kt + 1) * 128],
                        waa_sb,
                        start=(i == 0), stop=(i == 5),
                    )
                psv3 = psv[:, 0:384].rearrange("p (t c) -> p t c", c=64)
                scr = work.tile([128, 512], f32, tag="scr")
                nc.vector.tensor_scalar_mul(scr[:, 0:384], psv[:, 0:384], float(aa))
                scr3 = scr[:, 0:384].rearrange("p (t c) -> p t c", c=64)
                nc.vector.tensor_max(v3[:, 6 * w : 6 * w + 6, 0:64], scr3, psv3)

            def fill_e2(b0, nb):
                # e2 col blocks b0..b0+nb-1 (row-tiled trios; different
                # partitions may share the bank)
                pse = ps_f.tile([128, 512], f32, tag="fill")
                for m in range(nb):
                    for j in range(R):
                        kt = R * (b0 + m) + j
                        nc.tensor.matmul(
                            pse[32 * j : 32 * (j + 1), m * 128 : (m + 1) * 128],
                            w2a_sb,
                            xb_sb[:, kt * 128 : (kt + 1) * 128],
                            start=True, stop=True,
                        )
                prelu(
                    e2_sb[:, b0 * 128 : (b0 + nb) * 128],
                    pse[0:96, 0 : nb * 128],
                    a2,
                    pool_eng=True,
                )

            def fill_e1(off, n):
                pse = ps_f.tile([128, 512], f32, tag="fill")
                nc.tensor.matmul(
                    pse[:, 0:n], w1r_sb, xa_sb[:, off : off + n],
                    start=True, stop=True,
                )
                prelu(e1_sb[:, off : off + n], pse[:, 0:n], a1, pool_eng=True)

            # schedule: global iteration t (24 per main block).
            # v units (needed by PV in block 0) on evens, e2 units (needed
            # by QK in block 0) on odds, e1 blocks in the free late slots.
            # v unit 0 runs in the preamble shadow so the in-loop filler
            # chain (claim -> prelu -> next claim) starts pre-warmed
            fill_v(0)

            fillers = {}
            for w in range(1, 12):
                fillers.setdefault(2 * w - 2, []).append(lambda w=w: fill_v(w))
            for u in range(11):
                nb = 2 if u < 10 else 1
                fillers.setdefault(2 * u + 1, []).append(
                    lambda u=u, nb=nb: fill_e2(3 + 2 * u, nb)
                )
            # e1 blocks spread clear of the block-boundary DVE crunch
            fillers.setdefault(23, []).append(lambda: fill_e1(512, 512))
            fillers.setdefault(27, []).append(lambda: fill_e1(1024, 512))
            fillers.setdefault(31, []).append(lambda: fill_e1(1536, 512))
            fillers.setdefault(35, []).append(lambda: fill_e1(2048, 256))

            # --- main attention blocks (nq=512), software-pipelined ------
            for bi in range(NMAIN):
                off = 512 * bi
                po = ps_o.tile([C + 1, 512], f32, tag="po")
                pt_prev = None
                g_prev = -1

                def emit_pv(ptx, gx, po=po):
                    for j in range(R):
                        kt = R * gx + j
                        nc.tensor.matmul(
                            po[:, 0:512],
                            v_all[:, kt * 65 : (kt + 1) * 65],
                            ptx[:, j * 512 : (j + 1) * 512],
                            start=(kt == 0), stop=(kt == NKT - 1),
                        )

                for g in range(NG):
                    t = bi * NG + g
                    pss = ps.tile([128, 1536], f32, tag="ps")
                    for j in range(R):
                        kt = R * g + j
                        nc.tensor.matmul(
                            pss[:, j * 512 : (j + 1) * 512],
                            e2_sb[
                                32 * (kt % 3) : 32 * (kt % 3 + 1),
                                (kt // 3) * 128 : (kt // 3 + 1) * 128,
                            ],
                            e1_sb[32 * (kt % 3) : 32 * (kt % 3 + 1), off : off + 512],
                            start=True, stop=True,
                        )
                    pt = ptile.tile([128, 1536], bf16, tag="pt")
                    nc.scalar.activation(pt[:], pss[:], AF.Exp)
                    for fn in fillers.get(t, ()):
                        fn()
                    if pt_prev is not None:
                        emit_pv(pt_prev, g_prev)
                    pt_prev, g_prev = pt, g
                emit_pv(pt_prev, g_prev)

                # epilogue: out = po[0:64]/po[64] + xq. FIRST copy all of
                # po to SBUF in one DVE op - po is single-buffered, so the
                # next block's PV accumulation is gated on this read. The
                # recip + 0-stride broadcast DMA issue now, but the
                # DMA-gated mul and the add/store are DEFERRED into the
                # next block's iteration stream (the DVE queue is in-order,
                # so a waiting mul emitted here would jam the next block's
                # filler PReLUs behind it).
                poc = work.tile([C + 1, 512], f32, tag="poc")
                nc.vector.tensor_copy(poc[:], po[:])
                rec = work.tile([1, 512], f32, tag="rec")
                rb = work.tile([C, 512], f32, tag="rb")
                osb = work.tile([C, 512], f32, tag="osb")

                def epi_rec(rec=rec, poc=poc, rb=rb):
                    recip(rec[0:1, 0:512], poc[C : C + 1, 0:512])
                    rec_rep = rec[0:1, 0:512].rearrange("a (b c) -> a b c", b=1)
                    nc.gpsimd.dma_start(rb[:], rec_rep.to_broadcast((1, C, 512)))

                def epi_mul(osb=osb, rb=rb, poc=poc):
                    nc.vector.tensor_mul(osb[:], rb[:], poc[0:C, 0:512])

                def epi_add(osb=osb, off=off):
                    nc.vector.tensor_add(osb[:], osb[:], xa_sb[0:C, off : off + 512])
                    nc.sync.dma_start(out[:, off : off + 512], osb[:])

                t_next = (bi + 1) * NG
                fillers.setdefault(t_next + 2, []).append(epi_rec)
                fillers.setdefault(t_next + 6, []).append(epi_mul)
                fillers.setdefault(t_next + 8, []).append(epi_add)

            # --- tail block (queries 2048:2304, nq=256): SIX packed key
            # tiles per claim. Bank b holds the two tiles whose row group
            # is b (they serialize on the row group, so the bank never has
            # two concurrent groups); rounds r=0,1 issue as row-tiled trios.
            po = ps_o.tile([C + 1, 512], f32, tag="po")
            pt_prev = None
            sg_prev = -1

            def tail_col(kt):
                if TAIL_PACK:
                    return 512 * (kt % 3) + 256 * ((kt % 6) // 3)
                return 512 * (kt % 3)

            def emit_pv_tail(ptx, sgx):
                tpg = 6 if TAIL_PACK else 3
                for i in range(tpg):
                    kt = tpg * sgx + i
                    c0 = tail_col(kt)
                    nc.tensor.matmul(
                        po[:, 0:256],
                        v_all[:, kt * 65 : (kt + 1) * 65],
                        ptx[:, c0 : c0 + 256],
                        start=(kt == 0), stop=(kt == NKT - 1),
                    )

            NSG = 12 if TAIL_PACK else 24
            TPG = 6 if TAIL_PACK else 3
            for sg in range(NSG):
                pss = ps.tile([128, 1536], f32, tag="ps")
                for i in range(TPG):
                    kt = TPG * sg + i
                    c0 = tail_col(kt)
                    nc.tensor.matmul(
                        pss[:, c0 : c0 + 256],
                        e2_sb[
                            32 * (kt % 3) : 32 * (kt % 3 + 1),
                            (kt // 3) * 128 : (kt // 3 + 1) * 128,
                        ],
                        e1_sb[
                            32 * (kt % 3) : 32 * (kt % 3 + 1),
                            TAIL_OFF : TAIL_OFF + 256,
                        ],
                        start=True, stop=True,
                    )
                pt = ptile.tile([128, 1536], bf16, tag="pt")
                if TAIL_PACK:
                    nc.scalar.activation(pt[:], pss[:], AF.Exp)
                else:
                    pss3 = pss[:].rearrange("p (t c) -> p t c", c=512)
                    pt3 = pt[:].rearrange("p (t c) -> p t c", c=512)
                    nc.scalar.activation(pt3[:, :, 0:256], pss3[:, :, 0:256], AF.Exp)
                for fn in fillers.get(NMAIN * NG + sg, ()):
                    fn()
                if pt_prev is not None:
                    emit_pv_tail(pt_prev, sg_prev)
                pt_prev, sg_prev = pt, sg
            emit_pv_tail(pt_prev, sg_prev)

            # tail epilogue, split in halves so the DVE/PE/DMA chains
            # pipeline; reciprocal broadcast via an idle-PE K=1 matmul
            # (a broadcast DMA would add ~4us of un-hidden queue latency).
            # tail epilogue in halves (one engine does all the work, so
            # finer slicing only adds per-slice fixed costs)
            hn = 128
            rbps = []
            for h in range(2):
                ho = h * hn
                rec = work.tile([1, 512], f32, tag="rec")
                nc.vector.reciprocal(rec[0:1, 0:hn], po[C : C + 1, ho : ho + hn])
                rbp = ps.tile([128, 1536], f32, tag="ps")
                nc.tensor.matmul(
                    rbp[0:C, 0:hn], ones_sb[:], rec[0:1, 0:hn],
                    start=True, stop=True,
                )
                rbps.append(rbp)
            for h in range(2):
                ho = h * hn
                rb = work.tile([C, 512], f32, tag="rb")
                nc.vector.tensor_copy(rb[:, 0:hn], rbps[h][0:C, 0:hn])
                osb = work.tile([C, 512], f32, tag="osb")
                nc.vector.tensor_mul(osb[:, 0:hn], rb[:, 0:hn], po[0:C, ho : ho + hn])
                nc.vector.tensor_add(
                    osb[:, 0:hn], osb[:, 0:hn],
                    xa_sb[0:C, TAIL_OFF + ho : TAIL_OFF + ho + hn],
                )
                nc.sync.dma_start(
                    out[:, TAIL_OFF + ho : TAIL_OFF + ho + hn], osb[:, 0:hn]
                )
    nc.finalize()
    return nc


def run(inputs: dict, trace: bool = False, tmpdir: str | None = None):
    """Build, compile and run on 8 cores; returns (output, BassKernelResults)."""
    _ensure_ntff_hook()
    from concourse.bass_utils import run_bass_kernel_spmd

    inputa = np.asarray(inputs["inputa"], dtype=np.float32)
    inputb = np.asarray(inputs["inputb"], dtype=np.float32)
    w1 = np.asarray(inputs["w1"], dtype=np.float32)
    b1 = np.asarray(inputs["b1"], dtype=np.float32)
    w2 = np.asarray(inputs["w2"], dtype=np.float32)
    b2 = np.asarray(inputs["b2"], dtype=np.float32)
    wa = np.asarray(inputs["wa"], dtype=np.float32)
    ba = np.asarray(inputs["ba"], dtype=np.float32)
    a1 = float(np.asarray(inputs["a1"]).reshape(-1)[0])
    a2 = float(np.asarray(inputs["a2"]).reshape(-1)[0])
    aa = float(np.asarray(inputs["aa"]).reshape(-1)[0])

    N, Cc, H, W = inputa.shape
    assert (N, Cc, H * W) == (2, C, HW), inputa.shape
    chunks_per_batch = NCORES // N  # 4

    import ml_dtypes

    bf = ml_dtypes.bfloat16

    xa_n = inputa.reshape(N, C, HW)
    xb_n = inputb.reshape(N, C, HW)

    def aug65(x):
        """[64, HW] -> [65, HW] bf16 with a ones row at 64."""
        p = np.empty((C + 1, x.shape[1]), np.float32)
        p[:C] = x
        p[C] = 1.0
        return p.astype(bf)

    def wpad(wt, b, rep=1):
        """[64, M] weights^T + bias row at 64; optional column replication
        for the row-tiled QK stationary layout."""
        m = wt.shape[1]
        p = np.empty((C + 1, m * rep), np.float32)
        for r in range(rep):
            p[:C, r * m : (r + 1) * m] = wt
            p[C, r * m : (r + 1) * m] = b
        return p.astype(bf)

    wts_aug = np.concatenate(
        [wpad(w1.T, b1, rep=4), wpad(w2.T, b2), wpad(wa.T, ba)], axis=1
    )  # [65, 224]

    in_maps = []
    for core in range(NCORES):
        b, chunk = divmod(core, chunks_per_batch)
        qoff = chunk * QCH
        # rotate keys/values so this core's queries are columns 0:QCH
        # (softmax over keys is invariant to the key order)
        rot = np.concatenate([xa_n[b][:, qoff:], xa_n[b][:, :qoff]], axis=1)
        rot_b = np.concatenate([xb_n[b][:, qoff:], xb_n[b][:, :qoff]], axis=1)
        in_maps.append({"xa": aug65(rot), "xb": aug65(rot_b), "wts": wts_aug})

    nc = build_program(a1, a2, aa)
    res = run_bass_kernel_spmd(
        nc, in_maps, list(range(NCORES)), trace=trace, tmpdir=tmpdir
    )

    out = np.empty((N, C, HW), np.float32)
    for core in range(NCORES):
        b, chunk = divmod(core, chunks_per_batch)
        out[b, :, chunk * QCH : (chunk + 1) * QCH] = res.results[core]["out"]
    return out.reshape(N, C, H, W), res


def kernel(**inputs) -> np.ndarray:
    out, _ = run(inputs, trace=False)
    return out
